# revision 1
# baseline (speedup 1.0000x reference)
"""Adaptive mean thresholding (11x11 box mean, replicate border, C=2, INV)
on 8 trn2 NeuronCores. Batch data-parallel: 16 images of [512,512] per core.

Algorithm per core:
  The separable 11x11 box *sum* S = F @ I @ F^T with F the 512x512 banded
  (integer-count) filter matrix that encodes replicate-border clamping.
  Both 1D passes run on TensorE with the *data chunk as lhsT* (stationary)
  and constant F^T band-windows as rhs (moving), so each pass transposes
  the data; two passes land back in the original orientation with zero
  explicit transposes:
     pass1: VT[w, h'] = sum_h I[h, w] * FT[h, h']      (lhsT = I chunks)
     pass2: U[h', w'] = sum_w VT[w, h'] * FT[w, w']    (lhsT = VT chunks)
  Matmuls run in fp16 (10-bit mantissa, same as tf32, but fast weight
  load) with fp32 PSUM accumulation; inputs are mean-centered (free bias
  folds on existing ACT copies) so fp16 quantization error stays ~2e-3
  relative on the box mean. The threshold compare is
     out = 255 * (S >= 121*I + 242)   (exactly: 0 if I > S/121 - 2 else 255)
  done as one DVE scalar_tensor_tensor pass (d = 121*I - U', U' read from
  PSUM) and one DVE tensor_scalar dual-op (out = 255 * (d <= thr)).
  Banded-k0: start=True clears has_written for the whole PSUM bank, so all
  four accumulation matmuls use the exact ~133-wide band windows (2168
  instead of 3684 PE cycles per pass).
  DMA: loads on the SP HWDGE ring; stores on the ACT HWDGE ring with
  trigger emission delayed 2 images so a not-yet-ready store never
  head-of-line blocks ACT's FIFO. HWDGE stores sidestep the SWDGE hazard
  (a 2-port DVE op locks GpSimd out of the shared SBUF port pair,
  starving Q7 descriptor generation). 8 image buffers keep ~8 loads in
  flight. Measured DMA envelope (loads+stores, no compute) ~99 us vs
  94 us theoretical (33.6 MB @ 358 GB/s); full kernel ~106 us in the
  same measurement windows (was 135-160 us before this tuning).
"""

import sys

for p in ("/opt/trn_rl_repo", "/opt/trn_rl_repo/concourse"):
    if p not in sys.path:
        sys.path.insert(0, p)

import numpy as np

import concourse.bass as bass
import concourse.bacc as bacc
import concourse.mybir as mybir
import concourse.tile as tile
from concourse.bass_utils import run_bass_kernel_spmd

F32 = mybir.dt.float32
F16 = mybir.dt.float16

N_CORES = 8
B_PER_CORE = 16
H = W = 512
K = 11
PAD = K // 2
CONSTANT = 2.0
MAXVAL = 255.0
# which engine issues output DMAs ("scalar" -> qActDynamicHW ring,
# "gpsimd" -> SWDGE, "sync" -> qSPDynamicHW shared with loads).
# scalar+STORE_DELAY wins: HWDGE is immune to the DVE-2-port/SWDGE
# descriptor-starvation hazard, and the delay keeps ACT's FIFO from
# head-of-line blocking compute behind a not-yet-ready store trigger.
OUT_DMA_ENGINE = "scalar"
# DMA spread: cycles of engines per image index for loads/stores; None falls
# back to sync-loads / OUT_DMA_ENGINE-stores. More distinct rings engaged =>
# larger share of the SDMA packet round-robin when the chip is contended.
LOAD_ENGINES = None
STORE_ENGINES = None
SPLIT_DMA = False
# tail structure: "psum_dve" = DVE is_ge reads U from PSUM directly;
# "usb_act" = ACT evacuates U to SBUF first (DVE reads SBUF at 1x-fast init)
TAIL_MODE = "stt"
# engine for the t2 = 121*I + bias pass
T2_ENGINE = "vector"
# engine for the final *255 scale
SCALE_ENGINE = "scalar"

# rhs/psum windows per 128-block of the contraction dim; window k must
# contain the band [128k-5, 128k+133). Window 0 spans the full bank so the
# single start=True matmul initializes every element (PSUM has_written is
# cleared bank-wide by start=True); windows 1-3 accumulate, 256 wide to
# keep float32r at full rate (needs output free dim >= 256).
# default: 256-wide accumulate windows (f32r-era layout, also fine for fp16)
# banded: exact band windows [128k-5, 128k+133) - fp16 has no N>=256 rule
BANDED = True
# banded k=0 too: start=True clears has_written for the WHOLE bank, so the
# k=0 matmul can be banded [0,133) instead of full-width 512; elements not
# covered by k=0 are first-touched by k=1..3 (has_written=0 -> overwrite).
BANDED_K0 = True
# probe mode: skip all compute, store the loaded tile back (DMA envelope test)
DMA_ONLY = False
# probe-only: use 8KB-contiguous-per-partition descriptors instead of 2KB
DMA_CONTIG = False
# v2 pipeline: k-major matmul order, per-block imgr/evac, per-t tail+store.
# Shortens the per-image dependency chain (less pipeline fill/drain).
PIPE_V2 = False
# final scale op: "ts" = tensor_scalar dual-op (fast 2x_2P but LOCKS GpSimd
# out of the shared SBUF port pair while running -> starves SWDGE stores);
# "stt255" = scalar_tensor_tensor vs a 255-const tile (1x, never contends)
TS_MODE = "ts"
# emit image b's store at iteration b+STORE_DELAY (HWDGE store engines
# consume triggers in FIFO order; a ready trigger never head-of-line blocks)
STORE_DELAY = 2
# split the stt/ts/store tail into two 1024-wide halves per image: the
# first half's store can stream while the second half computes, and the
# pipeline drain chain shortens (HWDGE stores make the extra DMA free)
TAIL_HALVES = False
IMG_BUFS = 8
OUT_BUFS = 7
MID_BUFS = 3


def _window_layout():
    if BANDED and BANDED_K0:
        win = (0, 123, 251, 379)
        widths = (133, 138, 138, 133)
    elif BANDED:
        win = (0, 123, 251, 379)
        widths = (512, 138, 138, 133)
    else:
        win = (0, 64, 192, 256)
        widths = (512, 256, 256, 256)
    off = [0]
    for w in widths[:-1]:
        off.append(off[-1] + w)
    return win, widths, tuple(off), off[-1] + widths[-1]


WIN, WIDTHS, FTW_OFF, FTW_TOTAL = _window_layout()

# V = vertical 11-sum of U[0,255] pixels; center to cut tf32 quant error.
VCENTER = 11 * 127.5  # 1402.5
# out = 255 iff S >= 121*I + 242. With pass2 input centered:
#   U' = S - 11*VCENTER  =>  S >= 121*I + 242  <=>  U' >= 121*I + 242 - 11*VCENTER
T2_BIAS = 242.0 - 11 * VCENTER  # -15185.5


def _filter_matrix() -> np.ndarray:
    """F[o, i] = number of taps of output o's clamped window hitting input i."""
    F = np.zeros((H, H), dtype=np.float64)
    for o in range(H):
        for d in range(-PAD, PAD + 1):
            F[o, min(max(o + d, 0), H - 1)] += 1.0
    return F


def _ftw_windows() -> np.ndarray:
    """[128, FTW_TOTAL]: FT[128k:128(k+1), WIN[k]:WIN[k]+WIDTHS[k]], concat."""
    WIN, WIDTHS, FTW_OFF, FTW_TOTAL = _window_layout()
    FT = _filter_matrix().T
    tiles = [
        FT[128 * k : 128 * (k + 1), WIN[k] : WIN[k] + WIDTHS[k]] for k in range(4)
    ]
    return np.ascontiguousarray(np.concatenate(tiles, axis=1)).astype(np.float16)


class _nullcontext:
    def __enter__(self):
        return None

    def __exit__(self, *a):
        return False


def _emit_images(nc, tc, pools, img_d, out_d, ftw, c255=None):
    WIN, WIDTHS, FTW_OFF, FTW_TOTAL = _window_layout()
    (img_pool, imgr_pool, vt_pool, t2_pool, usb_pool, c01_pool, out_pool,
     vtps_pool, ups_pool) = pools
    load_cycle = LOAD_ENGINES or ("sync",)
    store_cycle = STORE_ENGINES or (OUT_DMA_ENGINE,)
    pending = {}

    def queue_store(b, outt, hh=None):
        dst = out_d[b * H : (b + 1) * H, :].rearrange("(t p) w -> p t w", p=128)
        st = getattr(nc, store_cycle[b % len(store_cycle)])
        if hh is None:
            item = (st, dst, outt[:].rearrange("p (t w) -> p t w", t=4))
        else:
            item = (st, dst[:, 2 * hh : 2 * hh + 2, :],
                    outt[:].rearrange("p (t w) -> p t w", t=2))
        if STORE_DELAY == 0:
            item[0].dma_start(item[1], item[2])
        else:
            pending.setdefault(b, []).append(item)

    def emit_due_store(b):
        for st, dst, src in pending.pop(b - STORE_DELAY, []):
            st.dma_start(dst, src)

    for b in range(B_PER_CORE):
        img = img_pool.tile([128, 4 * W], F32)
        if DMA_ONLY and DMA_CONTIG:
            src = img_d[b * H : (b + 1) * H, :].rearrange(
                "(p t) w -> p t w", p=128
            )
        else:
            src = img_d[b * H : (b + 1) * H, :].rearrange(
                "(t p) w -> p t w", p=128
            )
        ld = getattr(nc, load_cycle[b % len(load_cycle)])
        if SPLIT_DMA:
            ld2 = getattr(nc, load_cycle[(b + 1) % len(load_cycle)])
            dstap = img[:].rearrange("p (t w) -> p t w", t=4)
            ld.dma_start(dstap[:, 0:2, :], src[:, 0:2, :])
            ld2.dma_start(dstap[:, 2:4, :], src[:, 2:4, :])
        else:
            ld.dma_start(img[:].rearrange("p (t w) -> p t w", t=4), src)
        emit_due_store(b)

        if DMA_ONLY:
            if DMA_CONTIG:
                dst = out_d[b * H : (b + 1) * H, :].rearrange(
                    "(p t) w -> p t w", p=128
                )
            else:
                dst = out_d[b * H : (b + 1) * H, :].rearrange(
                    "(t p) w -> p t w", p=128
                )
            st = getattr(nc, store_cycle[b % len(store_cycle)])
            st.dma_start(dst, img[:].rearrange("p (t w) -> p t w", t=4))
            continue

        # fp16 copy of the image for the pass-1 matmuls, centered so the
        # fp16 quantization error is minimal; the filter weights sum to 11
        # per output, so pass-1 output is exactly V - 11*127.5 = V - VCENTER
        # and the evacuation needs no further centering. The threshold
        # compare keeps reading the exact fp32 image.
        img_r = imgr_pool.tile([128, 4 * W], F16)
        nc.scalar.activation(
            img_r[:], img[:], mybir.ActivationFunctionType.Copy, bias=-127.5
        )

        # pass 1: VT[wblk j] <- sum over row-blocks k of I-chunk^T @ FTwin
        vt_ps = vtps_pool.tile([128, 4 * W], F32)
        for j in range(4):
            for k in range(4):
                nc.tensor.matmul(
                    vt_ps[:, j * 512 + WIN[k] : j * 512 + WIN[k] + WIDTHS[k]],
                    img_r[:, k * 512 + j * 128 : k * 512 + j * 128 + 128],
                    ftw[:, FTW_OFF[k] : FTW_OFF[k] + WIDTHS[k]],
                    start=(k == 0),
                    stop=(k == 3),
                )

        # evacuate PSUM (already centered by the img_r bias)
        vt_sb = vt_pool.tile([128, 4 * W], F16)
        nc.scalar.activation(
            vt_sb[:], vt_ps[:], mybir.ActivationFunctionType.Copy
        )

        # pass 2: U[hblk t] <- sum over col-blocks k of VT-chunk^T @ FTwin
        u_ps = ups_pool.tile([128, 4 * W], F32)
        for t in range(4):
            for k in range(4):
                nc.tensor.matmul(
                    u_ps[:, t * 512 + WIN[k] : t * 512 + WIN[k] + WIDTHS[k]],
                    vt_sb[:, k * 512 + t * 128 : k * 512 + t * 128 + 128],
                    ftw[:, FTW_OFF[k] : FTW_OFF[k] + WIDTHS[k]],
                    start=(k == 0),
                    stop=(k == 3),
                )

        # threshold: out = 255 * (U' >= 121*I + T2_BIAS)
        if TAIL_MODE == "stt":
            # fused: d = (I*121) - U'  (one DVE pass, PSUM operand), then
            # out = 255 * (d <= -T2_BIAS)  (one DVE dual-op pass)
            if TAIL_HALVES:
                for hh in range(2):
                    sl = slice(hh * 2 * W, (hh + 1) * 2 * W)
                    d = c01_pool.tile([128, 2 * W], F32)
                    nc.vector.scalar_tensor_tensor(
                        d[:], img[:, sl], 121.0, u_ps[:, sl],
                        mybir.AluOpType.mult, mybir.AluOpType.subtract,
                    )
                    outt = out_pool.tile([128, 2 * W], F32)
                    nc.vector.tensor_scalar(
                        outt[:], d[:], -T2_BIAS, MAXVAL,
                        mybir.AluOpType.is_le, mybir.AluOpType.mult,
                    )
                    queue_store(b, outt, hh)
                continue
            d = c01_pool.tile([128, 4 * W], F32)
            nc.vector.scalar_tensor_tensor(
                d[:], img[:], 121.0, u_ps[:],
                mybir.AluOpType.mult, mybir.AluOpType.subtract,
            )
            outt = out_pool.tile([128, 4 * W], F32)
            if TS_MODE == "stt255":
                # 1x-mode op (two tensor reads): never grabs the shared
                # SBUF port pair, so SWDGE descriptor gen is never starved
                nc.vector.scalar_tensor_tensor(
                    outt[:], d[:], -T2_BIAS, c255[:],
                    mybir.AluOpType.is_le, mybir.AluOpType.mult,
                )
            else:
                nc.vector.tensor_scalar(
                    outt[:], d[:], -T2_BIAS, MAXVAL,
                    mybir.AluOpType.is_le, mybir.AluOpType.mult,
                )
            queue_store(b, outt)
            continue
        t2 = t2_pool.tile([128, 4 * W], F32)
        if T2_ENGINE == "scalar":
            nc.scalar.activation(
                t2[:],
                img[:],
                mybir.ActivationFunctionType.Copy,
                bias=T2_BIAS,
                scale=121.0,
            )
        else:
            nc.vector.tensor_scalar(
                t2[:], img[:], 121.0, T2_BIAS,
                mybir.AluOpType.mult, mybir.AluOpType.add,
            )
        if TAIL_MODE == "usb_act":
            u_sb = usb_pool.tile([128, 4 * W], F32)
            nc.scalar.activation(
                u_sb[:], u_ps[:], mybir.ActivationFunctionType.Copy
            )
            u_src = u_sb
        else:
            u_src = u_ps
        c01 = c01_pool.tile([128, 4 * W], F32)
        nc.vector.tensor_tensor(c01[:], u_src[:], t2[:], mybir.AluOpType.is_ge)
        outt = out_pool.tile([128, 4 * W], F32)
        if SCALE_ENGINE == "vector":
            nc.vector.tensor_scalar_mul(outt[:], c01[:], MAXVAL)
        else:
            nc.scalar.activation(
                outt[:], c01[:], mybir.ActivationFunctionType.Copy, scale=MAXVAL
            )

        queue_store(b, outt)

    for b in sorted(pending):
        for st, dst, src in pending[b]:
            st.dma_start(dst, src)
    pending.clear()


def _emit_images_v2(nc, tc, pools, img_d, out_d, ftw):
    """k-major pipeline: imgr cast per k-block feeds pass-1 k-sweeps, evac
    per j-block feeds pass-2 k-sweeps, tail + store per t-block. Shorter
    load->store chain and smoother store issue than the v1 whole-image
    stages."""
    WIN, WIDTHS, FTW_OFF, FTW_TOTAL = _window_layout()
    (img_pool, imgr_pool, vt_pool, t2_pool, usb_pool, c01_pool, out_pool,
     vtps_pool, ups_pool) = pools
    load_cycle = LOAD_ENGINES or ("sync",)
    store_cycle = STORE_ENGINES or (OUT_DMA_ENGINE,)
    for b in range(B_PER_CORE):
        img = img_pool.tile([128, 4 * W], F32)
        src = img_d[b * H : (b + 1) * H, :].rearrange("(t p) w -> p t w", p=128)
        ld = getattr(nc, load_cycle[b % len(load_cycle)])
        ld.dma_start(img[:].rearrange("p (t w) -> p t w", t=4), src)

        # fp16 centered image cast, one ACT op per k-block so pass-1 can
        # start after the first block instead of the full image
        img_r = imgr_pool.tile([128, 4 * W], F16)
        for k in range(4):
            nc.scalar.activation(
                img_r[:, k * 512 : (k + 1) * 512],
                img[:, k * 512 : (k + 1) * 512],
                mybir.ActivationFunctionType.Copy,
                bias=-127.5,
            )

        # pass 1, k-major: consume img_r block k across all 4 j windows
        vt_ps = vtps_pool.tile([128, 4 * W], F32)
        for k in range(4):
            for j in range(4):
                nc.tensor.matmul(
                    vt_ps[:, j * 512 + WIN[k] : j * 512 + WIN[k] + WIDTHS[k]],
                    img_r[:, k * 512 + j * 128 : k * 512 + j * 128 + 128],
                    ftw[:, FTW_OFF[k] : FTW_OFF[k] + WIDTHS[k]],
                    start=(k == 0),
                    stop=(k == 3),
                )

        # evac per j-block (pass-2 consumes vt_sb block k=j early)
        vt_sb = vt_pool.tile([128, 4 * W], F16)
        for j in range(4):
            nc.scalar.activation(
                vt_sb[:, j * 512 : (j + 1) * 512],
                vt_ps[:, j * 512 : (j + 1) * 512],
                mybir.ActivationFunctionType.Copy,
            )

        # pass 2, k-major
        u_ps = ups_pool.tile([128, 4 * W], F32)
        for k in range(4):
            for t in range(4):
                nc.tensor.matmul(
                    u_ps[:, t * 512 + WIN[k] : t * 512 + WIN[k] + WIDTHS[k]],
                    vt_sb[:, k * 512 + t * 128 : k * 512 + t * 128 + 128],
                    ftw[:, FTW_OFF[k] : FTW_OFF[k] + WIDTHS[k]],
                    start=(k == 0),
                    stop=(k == 3),
                )

        # tail + store per t-block: d = 121*I - U', out = 255*(d <= -T2_BIAS)
        for t in range(4):
            d = c01_pool.tile([128, W], F32)
            nc.vector.scalar_tensor_tensor(
                d[:], img[:, t * 512 : (t + 1) * 512], 121.0,
                u_ps[:, t * 512 : (t + 1) * 512],
                mybir.AluOpType.mult, mybir.AluOpType.subtract,
            )
            outt = out_pool.tile([128, W], F32)
            nc.vector.tensor_scalar(
                outt[:], d[:], -T2_BIAS, MAXVAL,
                mybir.AluOpType.is_le, mybir.AluOpType.mult,
            )
            st = getattr(nc, store_cycle[(4 * b + t) % len(store_cycle)])
            st.dma_start(
                out_d[b * H + t * 128 : b * H + (t + 1) * 128, :], outt[:]
            )


def _build_nc(reps: int = 1) -> bass.Bass:
    global WIN, WIDTHS, FTW_OFF, FTW_TOTAL
    WIN, WIDTHS, FTW_OFF, FTW_TOTAL = _window_layout()
    nc = bacc.Bacc()
    img_d = nc.declare_dram_parameter(
        "image", [B_PER_CORE * H, W], F32, isOutput=False
    )
    ftw_d = nc.declare_dram_parameter(
        "ftw", [128, FTW_TOTAL], F16, isOutput=False
    )
    out_d = nc.declare_dram_parameter("out", [B_PER_CORE * H, W], F32, isOutput=True)

    with tile.TileContext(nc) as tc:
        with (
            tc.tile_pool(name="const", bufs=1) as const_pool,
            tc.tile_pool(name="img", bufs=IMG_BUFS) as img_pool,
            tc.tile_pool(name="imgr", bufs=MID_BUFS) as imgr_pool,
            tc.tile_pool(name="vt", bufs=MID_BUFS) as vt_pool,
            tc.tile_pool(name="t2", bufs=MID_BUFS) as t2_pool,
            tc.tile_pool(name="usb", bufs=2) as usb_pool,
            tc.tile_pool(name="c01", bufs=(8 if PIPE_V2 else MID_BUFS)) as c01_pool,
            tc.tile_pool(name="outp", bufs=OUT_BUFS) as out_pool,
            tc.tile_pool(name="vtps", bufs=1, space="PSUM") as vtps_pool,
            tc.tile_pool(name="ups", bufs=1, space="PSUM") as ups_pool,
        ):
            pools = (img_pool, imgr_pool, vt_pool, t2_pool, usb_pool,
                     c01_pool, out_pool, vtps_pool, ups_pool)
            ftw = const_pool.tile([128, FTW_TOTAL], F16)
            nc.sync.dma_start(ftw[:], ftw_d[:])
            c255 = None
            if TS_MODE == "stt255":
                c255 = const_pool.tile([128, 4 * W], F32)
                nc.vector.memset(c255[:], MAXVAL)

            if reps > 1:
                # benchmark mode: run the whole pipeline reps times inside
                # the NEFF so per-call dispatch overhead amortizes away
                loop_ctx = tc.For_i(0, reps, 1)
            else:
                loop_ctx = _nullcontext()
            with loop_ctx:
                if PIPE_V2:
                    _emit_images_v2(nc, tc, pools, img_d, out_d, ftw)
                else:
                    _emit_images(nc, tc, pools, img_d, out_d, ftw, c255)

    nc.compile()
    return nc


_NC_CACHE = None


def _get_nc() -> bass.Bass:
    global _NC_CACHE
    if _NC_CACHE is None:
        _NC_CACHE = _build_nc()
    return _NC_CACHE


def kernel(image: np.ndarray) -> np.ndarray:
    assert image.shape == (128, H, W, 1), image.shape
    img = np.ascontiguousarray(image.reshape(128, H, W).astype(np.float32))
    ftw = _ftw_windows()

    in_maps = []
    for c in range(N_CORES):
        shard = img[c * B_PER_CORE : (c + 1) * B_PER_CORE].reshape(
            B_PER_CORE * H, W
        )
        in_maps.append({"image": np.ascontiguousarray(shard), "ftw": ftw})

    nc = _get_nc()
    res = run_bass_kernel_spmd(nc, in_maps, core_ids=list(range(N_CORES)))
    shards = [
        res.results[c]["out"].reshape(B_PER_CORE, H, W, 1) for c in range(N_CORES)
    ]
    return np.concatenate(shards, axis=0).astype(np.float32)



# revision 2
# speedup vs baseline: 1.0781x; 1.0781x over previous
"""Adaptive mean thresholding (11x11 box mean, replicate border, C=2, INV)
on 8 trn2 NeuronCores. Batch data-parallel: 16 images of [512,512] per core.

HBM-traffic-minimized version (the previous f32-in/f32-out kernel was
DMA-bound at ~99us envelope for 33.6MB/core):
  - Input ships as fp16 of the *centered* image x = fp16(I - 127.5)
    (2B/px, host-side cast). Centering keeps fp16 quantization at
    ~0.01 gray-levels rms; a numpy simulation of the full quantized chain
    measures rel err 3.5e-3 vs the exact reference (gate is 2e-2).
  - Output ships as uint8 0/1 (1B/px); the host scales to 0.0/255.0 f32
    during the gather. Total traffic 12.6MB/core (~35us at 358GB/s).
  - Rows are pre-permuted on the host into the [p, (t, w)] SBUF layout
    (partition p holds rows {128t+p}), so every load descriptor is 4KB
    contiguous and every store descriptor 2KB contiguous per partition.

Algorithm per core (per image):
  Separable 11x11 box sum via two TensorE passes with the data chunk as
  lhsT (stationary) and banded FT windows as rhs; each pass transposes, so
  two passes land back in the input orientation with zero explicit
  transposes. Weights are *dyadic* scaled (F/8 and F/16, exact in fp16);
  the ACT PSUM-evacuation pass folds the exact 128/121 rescale plus the
  threshold bias (-32/11) in f32, so the final PSUM result is directly
  U = S'/121 - 2 in centered-image units and the tail is a single DVE op:
     out_u8 = (x <= U)            # 1 -> 255 case, 0 -> 0 case
  Engine cost per image: PE 2x16 banded matmuls (~2.7us), ACT one
  2048-elem evac (~2.0us), DVE one 2048-elem 1x tensor_tensor (~2.3us),
  DMA 6KB/partition (~2.2us). All four engines land just above the DMA
  envelope, ~2.3x faster than the f32 kernel.
  DMA: loads on the SP HWDGE ring; stores on the ACT HWDGE ring with
  trigger emission delayed 2 images so a not-yet-ready store never
  head-of-line blocks ACT's FIFO.
"""

import sys

for p in ("/opt/trn_rl_repo", "/opt/trn_rl_repo/concourse"):
    if p not in sys.path:
        sys.path.insert(0, p)

import numpy as np

import concourse.bass as bass
import concourse.bacc as bacc
import concourse.mybir as mybir
import concourse.tile as tile
from concourse.bass_utils import run_bass_kernel_spmd

F32 = mybir.dt.float32
F16 = mybir.dt.float16
U8 = mybir.dt.uint8

N_CORES = 8
B_PER_CORE = 16
H = W = 512
K = 11
PAD = K // 2

# evac: Vb = fp16((128/121) * V + BETA); U = sum F/16 * Vb = S'/121 - 2
EVAC_SCALE = 128.0 / 121.0
EVAC_BIAS = -32.0 / 11.0

IMG_BUFS = 8
VT_BUFS = 3
OUT_BUFS = 6
STORE_DELAY = 2
LOAD_ENGINE = "sync"
STORE_ENGINE = "scalar"

# banded windows: window k must contain the band [128k-5, 128k+133).
# k=0 banded too: start=True clears has_written for the WHOLE psum bank.
WIN = (0, 123, 251, 379)
WIDTHS = (133, 138, 138, 133)
FTW_OFF = (0, 133, 271, 409)
FTW_TOTAL = 542


def _filter_matrix() -> np.ndarray:
    """F[o, i] = number of taps of output o's clamped window hitting input i."""
    F = np.zeros((H, H), dtype=np.float64)
    for o in range(H):
        for d in range(-PAD, PAD + 1):
            F[o, min(max(o + d, 0), H - 1)] += 1.0
    return F


def _ftw_windows() -> np.ndarray:
    """[128, 2*FTW_TOTAL]: FT/8 band windows then FT/16 band windows."""
    FT = _filter_matrix().T
    out = []
    for scale in (8.0, 16.0):
        tiles = [
            FT[128 * k : 128 * (k + 1), WIN[k] : WIN[k] + WIDTHS[k]] / scale
            for k in range(4)
        ]
        out.append(np.concatenate(tiles, axis=1))
    return np.ascontiguousarray(np.concatenate(out, axis=1)).astype(np.float16)


def prepare_shards(image: np.ndarray) -> list[dict[str, np.ndarray]]:
    """Full [128, 512, 512, 1] f32 image -> per-core input maps.

    Ships x = fp16(I - 127.5) row-permuted so partition p holds image rows
    {128t + p} as 4 contiguous 512-px chunks (4KB/partition descriptors).
    """
    img = image.reshape(128, H, W).astype(np.float32)
    x = (img - np.float32(127.5)).astype(np.float16)
    # [B, H, W] -> [B, 4, 128, 512] -> [B, 128, 4, 512] -> [B*128, 2048]
    xp = np.ascontiguousarray(
        x.reshape(128, 4, 128, W).transpose(0, 2, 1, 3)
    ).reshape(128, 128, 4 * W)
    ftw = _ftw_windows()
    in_maps = []
    for c in range(N_CORES):
        shard = xp[c * B_PER_CORE : (c + 1) * B_PER_CORE].reshape(
            B_PER_CORE * 128, 4 * W
        )
        in_maps.append({"image": np.ascontiguousarray(shard), "ftw": ftw})
    return in_maps


def postprocess(shards: list[np.ndarray]) -> np.ndarray:
    """Per-core uint8 0/1 outputs [B*128, 2048] -> full f32 0/255 output."""
    u8 = np.concatenate(
        [s.reshape(B_PER_CORE, 128, 4, W) for s in shards], axis=0
    )
    # [128img, 128p, 4t, 512] -> [128img, 4t, 128p, 512] -> [128, 512, 512]
    out01 = u8.transpose(0, 2, 1, 3).reshape(128, H, W, 1)
    return out01.astype(np.float32) * np.float32(255.0)


class _nullcontext:
    def __enter__(self):
        return None

    def __exit__(self, *a):
        return False


def _emit_images(nc, tc, pools, img_d, out_d, ftw):
    img_pool, vt_pool, out_pool, vtps_pool, ups_pool = pools
    pending = {}

    def queue_store(b, outt):
        st = getattr(nc, STORE_ENGINE)
        item = (st, out_d[b * 128 : (b + 1) * 128, :], outt[:])
        if STORE_DELAY == 0:
            item[0].dma_start(item[1], item[2])
        else:
            pending.setdefault(b, []).append(item)

    def emit_due_store(b):
        for st, dst, src in pending.pop(b - STORE_DELAY, []):
            st.dma_start(dst, src)

    for b in range(B_PER_CORE):
        img = img_pool.tile([128, 4 * W], F16)
        ld = getattr(nc, LOAD_ENGINE)
        ld.dma_start(img[:], img_d[b * 128 : (b + 1) * 128, :])
        emit_due_store(b)

        # pass 1: VT[wblk j] <- sum over row-blocks k of x-chunk^T @ (FT/8)win
        vt_ps = vtps_pool.tile([128, 4 * W], F32)
        for j in range(4):
            for k in range(4):
                nc.tensor.matmul(
                    vt_ps[:, j * 512 + WIN[k] : j * 512 + WIN[k] + WIDTHS[k]],
                    img[:, k * 512 + j * 128 : k * 512 + j * 128 + 128],
                    ftw[:, FTW_OFF[k] : FTW_OFF[k] + WIDTHS[k]],
                    start=(k == 0),
                    stop=(k == 3),
                )

        # evacuate PSUM with the exact 128/121 rescale + threshold bias
        vt_sb = vt_pool.tile([128, 4 * W], F16)
        nc.scalar.activation(
            vt_sb[:], vt_ps[:], mybir.ActivationFunctionType.Copy,
            bias=EVAC_BIAS, scale=EVAC_SCALE,
        )

        # pass 2: U[hblk t] <- sum over col-blocks k of VT-chunk^T @ (FT/16)win
        u_ps = ups_pool.tile([128, 4 * W], F32)
        for t in range(4):
            for k in range(4):
                nc.tensor.matmul(
                    u_ps[:, t * 512 + WIN[k] : t * 512 + WIN[k] + WIDTHS[k]],
                    vt_sb[:, k * 512 + t * 128 : k * 512 + t * 128 + 128],
                    ftw[:, FTW_TOTAL + FTW_OFF[k] : FTW_TOTAL + FTW_OFF[k] + WIDTHS[k]],
                    start=(k == 0),
                    stop=(k == 3),
                )

        # tail: out = 1 iff x <= U  (host scales 1 -> 255.0)
        outt = out_pool.tile([128, 4 * W], U8)
        nc.vector.tensor_tensor(outt[:], img[:], u_ps[:], mybir.AluOpType.is_le)
        queue_store(b, outt)

    for b in sorted(pending):
        for st, dst, src in pending[b]:
            st.dma_start(dst, src)
    pending.clear()


def _build_nc(reps: int = 1) -> bass.Bass:
    nc = bacc.Bacc()
    img_d = nc.declare_dram_parameter(
        "image", [B_PER_CORE * 128, 4 * W], F16, isOutput=False
    )
    ftw_d = nc.declare_dram_parameter(
        "ftw", [128, 2 * FTW_TOTAL], F16, isOutput=False
    )
    out_d = nc.declare_dram_parameter(
        "out", [B_PER_CORE * 128, 4 * W], U8, isOutput=True
    )

    with tile.TileContext(nc) as tc:
        with (
            tc.tile_pool(name="const", bufs=1) as const_pool,
            tc.tile_pool(name="img", bufs=IMG_BUFS) as img_pool,
            tc.tile_pool(name="vt", bufs=VT_BUFS) as vt_pool,
            tc.tile_pool(name="outp", bufs=OUT_BUFS) as out_pool,
            tc.tile_pool(name="vtps", bufs=1, space="PSUM") as vtps_pool,
            tc.tile_pool(name="ups", bufs=1, space="PSUM") as ups_pool,
        ):
            pools = (img_pool, vt_pool, out_pool, vtps_pool, ups_pool)
            ftw = const_pool.tile([128, 2 * FTW_TOTAL], F16)
            nc.sync.dma_start(ftw[:], ftw_d[:])

            if reps > 1:
                loop_ctx = tc.For_i(0, reps, 1)
            else:
                loop_ctx = _nullcontext()
            with loop_ctx:
                _emit_images(nc, tc, pools, img_d, out_d, ftw)

    nc.compile()
    return nc


_NC_CACHE = None


def _get_nc() -> bass.Bass:
    global _NC_CACHE
    if _NC_CACHE is None:
        _NC_CACHE = _build_nc()
    return _NC_CACHE


def kernel(image: np.ndarray) -> np.ndarray:
    assert image.shape == (128, H, W, 1), image.shape
    in_maps = prepare_shards(image)
    nc = _get_nc()
    res = run_bass_kernel_spmd(nc, in_maps, core_ids=list(range(N_CORES)))
    return postprocess([res.results[c]["out"] for c in range(N_CORES)])


# revision 4
# speedup vs baseline: 1.9603x; 1.8184x over previous
"""Adaptive mean thresholding (11x11 box mean, replicate border, C=2, INV)
on 8 trn2 NeuronCores. Batch data-parallel: 16 images of [512,512] per core.

HBM-traffic-minimized version (the previous f32-in/f32-out kernel was
DMA-bound at ~99us envelope for 33.6MB/core):
  - Input ships as fp16 of the *centered* image x = fp16(I - 127.5)
    (2B/px, host-side cast). Centering keeps fp16 quantization at
    ~0.01 gray-levels rms; a numpy simulation of the full quantized chain
    measures rel err 3.5e-3 vs the exact reference (gate is 2e-2).
  - Output ships as uint8 0/1 (1B/px); the host scales to 0.0/255.0 f32
    during the gather. Total traffic 12.6MB/core (~35us at 358GB/s).
  - Rows are pre-permuted on the host into the [p, (t, w)] SBUF layout
    (partition p holds rows {128t+p}), so every load descriptor is 4KB
    contiguous and every store descriptor 2KB contiguous per partition.

Algorithm per core (per image):
  Separable 11x11 box sum via two TensorE passes with the data chunk as
  lhsT (stationary) and banded FT windows as rhs; each pass transposes, so
  two passes land back in the input orientation with zero explicit
  transposes. Weights are *dyadic* scaled (F/8 and F/16, exact in fp16);
  the ACT PSUM-evacuation pass folds the exact 128/121 rescale plus the
  threshold bias (-32/11) in f32, so the final PSUM result is directly
  U = S'/121 - 2 in centered-image units and the tail is a single DVE op:
     out_u8 = (x <= U)            # 1 -> 255 case, 0 -> 0 case
  Engine cost per image: PE 2x16 banded matmuls (~2.7us), ACT one
  2048-elem evac (~2.0us), DVE one 2048-elem 1x tensor_tensor (~2.3us),
  DMA 6KB/partition (~2.2us). All four engines land just above the DMA
  envelope, ~2.3x faster than the f32 kernel.
  DMA: loads on the SP HWDGE ring; stores on the ACT HWDGE ring with
  trigger emission delayed 2 images so a not-yet-ready store never
  head-of-line blocks ACT's FIFO.
"""

import sys

for p in ("/opt/trn_rl_repo", "/opt/trn_rl_repo/concourse"):
    if p not in sys.path:
        sys.path.insert(0, p)

import numpy as np

import concourse.bass as bass
import concourse.bacc as bacc
import concourse.mybir as mybir
import concourse.tile as tile
from concourse.bass_utils import run_bass_kernel_spmd

F32 = mybir.dt.float32
F16 = mybir.dt.float16
U8 = mybir.dt.uint8

N_CORES = 8
B_PER_CORE = 16
H = W = 512
K = 11
PAD = K // 2

# evac: Vb = fp16((128/121) * V + BETA); U = sum F/16 * Vb = S'/121 - 2
EVAC_SCALE = 128.0 / 121.0
EVAC_BIAS = -32.0 / 11.0

IMG_BUFS = 8
VT_BUFS = 3
OUT_BUFS = 6
STORE_DELAY = 2
LOAD_ENGINE = "sync"
STORE_ENGINE = "scalar"

# banded windows: window k must contain the band [128k-5, 128k+133).
# k=0 banded too: start=True clears has_written for the WHOLE psum bank.
WIN = (0, 123, 251, 379)
WIDTHS = (133, 138, 138, 133)
FTW_OFF = (0, 133, 271, 409)
FTW_TOTAL = 542


def _filter_matrix() -> np.ndarray:
    """F[o, i] = number of taps of output o's clamped window hitting input i."""
    F = np.zeros((H, H), dtype=np.float64)
    for o in range(H):
        for d in range(-PAD, PAD + 1):
            F[o, min(max(o + d, 0), H - 1)] += 1.0
    return F


def _ftw_windows() -> np.ndarray:
    """[128, 2*FTW_TOTAL]: FT/8 band windows then FT/16 band windows."""
    FT = _filter_matrix().T
    out = []
    for scale in (8.0, 16.0):
        tiles = [
            FT[128 * k : 128 * (k + 1), WIN[k] : WIN[k] + WIDTHS[k]] / scale
            for k in range(4)
        ]
        out.append(np.concatenate(tiles, axis=1))
    return np.ascontiguousarray(np.concatenate(out, axis=1)).astype(np.float16)


def prepare_shards(image: np.ndarray) -> list[dict[str, np.ndarray]]:
    """Full [128, 512, 512, 1] f32 image -> per-core input maps.

    Ships x = fp16(I - 127.5) row-permuted so partition p holds image rows
    {128t + p} as 4 contiguous 512-px chunks (4KB/partition descriptors).
    """
    img = image.reshape(128, H, W).astype(np.float32)
    x = (img - np.float32(127.5)).astype(np.float16)
    # [B, H, W] -> [B, 4, 128, 512] -> [B, 128, 4, 512] -> [B*128, 2048]
    xp = np.ascontiguousarray(
        x.reshape(128, 4, 128, W).transpose(0, 2, 1, 3)
    ).reshape(128, 128, 4 * W)
    ftw = _ftw_windows()
    in_maps = []
    for c in range(N_CORES):
        shard = xp[c * B_PER_CORE : (c + 1) * B_PER_CORE].reshape(
            B_PER_CORE * 128, 4 * W
        )
        in_maps.append({"image": np.ascontiguousarray(shard), "ftw": ftw})
    return in_maps


def postprocess(shards: list[np.ndarray]) -> np.ndarray:
    """Per-core uint8 0/1 outputs [B*128, 2048] -> full f32 0/255 output."""
    u8 = np.concatenate(
        [s.reshape(B_PER_CORE, 128, 4, W) for s in shards], axis=0
    )
    # [128img, 128p, 4t, 512] -> [128img, 4t, 128p, 512] -> [128, 512, 512]
    out01 = u8.transpose(0, 2, 1, 3).reshape(128, H, W, 1)
    return out01.astype(np.float32) * np.float32(255.0)


class _nullcontext:
    def __enter__(self):
        return None

    def __exit__(self, *a):
        return False


def _emit_images(nc, tc, pools, img_d, out_d, ftw):
    """One-image software pipeline: pass 2 + compare of image b-1 are emitted
    inside iteration b, so ACT's evac of image b overlaps PE's pass 2 of
    image b-1 and DVE's compares chase pass 2. PSUM is split into four
    2-bank half tiles (vtA/vtB/uA/uB x bufs=2 = 8 banks exactly) so no
    engine ever waits a full-image latency for a buffer."""
    img_pool, vt_pool, out_pool, vtps_pool, ups_pool = pools
    pending = {}

    def queue_store(b, outt):
        st = getattr(nc, STORE_ENGINE)
        item = (st, out_d[b * 128 : (b + 1) * 128, :], outt[:])
        if STORE_DELAY == 0:
            item[0].dma_start(item[1], item[2])
        else:
            pending.setdefault(b, []).append(item)

    def emit_due_store(b):
        for st, dst, src in pending.pop(b - STORE_DELAY, []):
            st.dma_start(dst, src)

    def pass1_half(img, vt_sb, half):
        """j-blocks {2*half, 2*half+1} -> one 2-bank PSUM tile -> ACT evac."""
        vt_ps = vtps_pool.tile([128, 2 * W], F32)
        for jj in range(2):
            j = 2 * half + jj
            for k in range(4):
                nc.tensor.matmul(
                    vt_ps[:, jj * 512 + WIN[k] : jj * 512 + WIN[k] + WIDTHS[k]],
                    img[:, k * 512 + j * 128 : k * 512 + j * 128 + 128],
                    ftw[:, FTW_OFF[k] : FTW_OFF[k] + WIDTHS[k]],
                    start=(k == 0),
                    stop=(k == 3),
                )
        nc.scalar.activation(
            vt_sb[:, half * 1024 : (half + 1) * 1024], vt_ps[:],
            mybir.ActivationFunctionType.Copy,
            bias=EVAC_BIAS, scale=EVAC_SCALE,
        )

    def pass2_half(img, vt_sb, outt, half):
        """t-blocks {2*half, 2*half+1} -> 2-bank PSUM tile -> DVE compare."""
        u_ps = ups_pool.tile([128, 2 * W], F32)
        for tt in range(2):
            t = 2 * half + tt
            for k in range(4):
                nc.tensor.matmul(
                    u_ps[:, tt * 512 + WIN[k] : tt * 512 + WIN[k] + WIDTHS[k]],
                    vt_sb[:, k * 512 + t * 128 : k * 512 + t * 128 + 128],
                    ftw[:, FTW_TOTAL + FTW_OFF[k] : FTW_TOTAL + FTW_OFF[k] + WIDTHS[k]],
                    start=(k == 0),
                    stop=(k == 3),
                )
        sl = slice(half * 1024, (half + 1) * 1024)
        nc.vector.tensor_tensor(
            outt[:, sl], img[:, sl], u_ps[:], mybir.AluOpType.is_le
        )

    prev = None  # (b, img, vt_sb) of the previous image
    for b in range(B_PER_CORE):
        img = img_pool.tile([128, 4 * W], F16)
        ld = getattr(nc, LOAD_ENGINE)
        ld.dma_start(img[:], img_d[b * 128 : (b + 1) * 128, :])
        emit_due_store(b)

        vt_sb = vt_pool.tile([128, 4 * W], F16)
        pass1_half(img, vt_sb, 0)
        pass1_half(img, vt_sb, 1)

        if prev is not None:
            pb, pimg, pvt = prev
            poutt = out_pool.tile([128, 4 * W], U8)
            pass2_half(pimg, pvt, poutt, 0)
            pass2_half(pimg, pvt, poutt, 1)
            queue_store(pb, poutt)
        prev = (b, img, vt_sb)

    pb, pimg, pvt = prev
    poutt = out_pool.tile([128, 4 * W], U8)
    pass2_half(pimg, pvt, poutt, 0)
    pass2_half(pimg, pvt, poutt, 1)
    queue_store(pb, poutt)

    for b in sorted(pending):
        for st, dst, src in pending[b]:
            st.dma_start(dst, src)
    pending.clear()


def _build_nc(reps: int = 1) -> bass.Bass:
    nc = bacc.Bacc()
    img_d = nc.declare_dram_parameter(
        "image", [B_PER_CORE * 128, 4 * W], F16, isOutput=False
    )
    ftw_d = nc.declare_dram_parameter(
        "ftw", [128, 2 * FTW_TOTAL], F16, isOutput=False
    )
    out_d = nc.declare_dram_parameter(
        "out", [B_PER_CORE * 128, 4 * W], U8, isOutput=True
    )

    with tile.TileContext(nc) as tc:
        with (
            tc.tile_pool(name="const", bufs=1) as const_pool,
            tc.tile_pool(name="img", bufs=IMG_BUFS) as img_pool,
            tc.tile_pool(name="vt", bufs=VT_BUFS) as vt_pool,
            tc.tile_pool(name="outp", bufs=OUT_BUFS) as out_pool,
            tc.tile_pool(name="vtps", bufs=2, space="PSUM") as vtps_pool,
            tc.tile_pool(name="ups", bufs=2, space="PSUM") as ups_pool,
        ):
            pools = (img_pool, vt_pool, out_pool, vtps_pool, ups_pool)
            ftw = const_pool.tile([128, 2 * FTW_TOTAL], F16)
            nc.sync.dma_start(ftw[:], ftw_d[:])

            if reps > 1:
                loop_ctx = tc.For_i(0, reps, 1)
            else:
                loop_ctx = _nullcontext()
            with loop_ctx:
                _emit_images(nc, tc, pools, img_d, out_d, ftw)

    nc.compile()
    return nc


_NC_CACHE = None


def _get_nc() -> bass.Bass:
    global _NC_CACHE
    if _NC_CACHE is None:
        _NC_CACHE = _build_nc()
    return _NC_CACHE


def kernel(image: np.ndarray) -> np.ndarray:
    assert image.shape == (128, H, W, 1), image.shape
    in_maps = prepare_shards(image)
    nc = _get_nc()
    res = run_bass_kernel_spmd(nc, in_maps, core_ids=list(range(N_CORES)))
    return postprocess([res.results[c]["out"] for c in range(N_CORES)])


# revision 17
# speedup vs baseline: 2.6754x; 1.3648x over previous
"""Adaptive mean thresholding (11x11 box mean, replicate border, C=2, INV)
on 8 trn2 NeuronCores. Batch data-parallel: 16 images of [512,512] per core.

HBM-traffic-minimized version (the previous f32-in/f32-out kernel was
DMA-bound at ~99us envelope for 33.6MB/core):
  - Input ships as fp16 of the *centered* image x = fp16(I - 127.5)
    (2B/px, host-side cast). Centering keeps fp16 quantization at
    ~0.01 gray-levels rms; a numpy simulation of the full quantized chain
    measures rel err 3.5e-3 vs the exact reference (gate is 2e-2).
  - Output ships as uint8 0/1 (1B/px); the host scales to 0.0/255.0 f32
    during the gather. Total traffic 12.6MB/core (~35us at 358GB/s).
  - Rows are pre-permuted on the host into the [p, (t, w)] SBUF layout
    (partition p holds rows {128t+p}), so every load descriptor is 4KB
    contiguous and every store descriptor 2KB contiguous per partition.

Algorithm per core (per image):
  Separable 11x11 box sum via two TensorE passes with the data chunk as
  lhsT (stationary) and banded FT windows as rhs; each pass transposes, so
  two passes land back in the input orientation with zero explicit
  transposes. Weights are *dyadic* scaled (F/8 and F/16, exact in fp16);
  the ACT PSUM-evacuation pass folds the exact 128/121 rescale plus the
  threshold bias (-32/11) in f32, so the final PSUM result is directly
  U = S'/121 - 2 in centered-image units and the tail is a single DVE op:
     out_u8 = (x <= U)            # 1 -> 255 case, 0 -> 0 case
  Schedule: one-image software pipeline (iteration b emits pass 1 of
  image b, then pass 2 + compare of image b-1) with PSUM split into four
  2-bank half tiles (vt x2 + u x2, bufs=2 each = 8 banks exactly), so PE
  streams matmuls back-to-back (~57 ns/MM measured, LDW fully hidden by
  FWL + the PE reorder window) while ACT evacs and DVE compares chase the
  halves. Measured ablations (shared device, medians): DMA envelope
  ~34us, loads+pass1 ~15us, all-but-compare ~29us, full ~33us quiet /
  ~40us typical. A single full-image u tile (one compare op) measures
  +20us - image b+1's pass 2 stalls on compare(b) - hence the halves.
  DMA: loads on the SP HWDGE ring; stores on the ACT HWDGE ring with
  trigger emission delayed 6 images (STORE_DELAY<=4 measures 6-12us
  slower: a store trigger whose compare hasn't finished occupies the
  8-deep strict-FIFO ACT queue and head-of-line blocks the next evac).
"""

import sys

for p in ("/opt/trn_rl_repo", "/opt/trn_rl_repo/concourse"):
    if p not in sys.path:
        sys.path.insert(0, p)

import numpy as np

import concourse.bass as bass
import concourse.bacc as bacc
import concourse.mybir as mybir
import concourse.tile as tile
from concourse.bass_utils import run_bass_kernel_spmd

F32 = mybir.dt.float32
F16 = mybir.dt.float16
U8 = mybir.dt.uint8

N_CORES = 8
B_PER_CORE = 16
H = W = 512
K = 11
PAD = K // 2

# evac: Vb = fp16((128/121) * V + BETA); U = sum F/16 * Vb = S'/121 - 2
EVAC_SCALE = 128.0 / 121.0
EVAC_BIAS = -32.0 / 11.0

IMG_BUFS = 12
VT_BUFS = 3
OUT_BUFS = 10
STORE_DELAY = 6
LOAD_ENGINE = "sync"
STORE_ENGINE = "scalar"
# diagnostic ablations (timing-only; outputs are garbage for any value
# other than "full"): "dma" = loads+stores only; "pe" = loads + both PE
# passes (pass 2 reads unwritten SBUF); "pe1" = loads + pass 1 only;
# "noact" = skip evac; "nodve" = skip compares; "nostore" = skip stores.
VARIANT = "full"
# 1 = single [128,2048] u PSUM tile + one full-image DVE compare (fewer DVE
# fixed overheads, but image b+1's pass 2 must wait on compare(b));
# 0 = two [128,1024] half tiles + two compares (default).
UPS_FULL = 0

# banded windows: window k must contain the band [128k-5, 128k+133).
# k=0 banded too: start=True clears has_written for the WHOLE psum bank.
WIN = (0, 123, 251, 379)
WIDTHS = (133, 138, 138, 133)
FTW_OFF = (0, 133, 271, 409)
FTW_TOTAL = 542


def _filter_matrix() -> np.ndarray:
    """F[o, i] = number of taps of output o's clamped window hitting input i."""
    F = np.zeros((H, H), dtype=np.float64)
    for o in range(H):
        for d in range(-PAD, PAD + 1):
            F[o, min(max(o + d, 0), H - 1)] += 1.0
    return F


def _ftw_windows() -> np.ndarray:
    """[128, 2*FTW_TOTAL]: FT/8 band windows then FT/16 band windows."""
    FT = _filter_matrix().T
    out = []
    for scale in (8.0, 16.0):
        tiles = [
            FT[128 * k : 128 * (k + 1), WIN[k] : WIN[k] + WIDTHS[k]] / scale
            for k in range(4)
        ]
        out.append(np.concatenate(tiles, axis=1))
    return np.ascontiguousarray(np.concatenate(out, axis=1)).astype(np.float16)


def prepare_shards(image: np.ndarray) -> list[dict[str, np.ndarray]]:
    """Full [128, 512, 512, 1] f32 image -> per-core input maps.

    Ships x = fp16(I - 127.5) row-permuted so partition p holds image rows
    {128t + p} as 4 contiguous 512-px chunks (4KB/partition descriptors).
    """
    img = image.reshape(128, H, W).astype(np.float32)
    x = (img - np.float32(127.5)).astype(np.float16)
    # [B, H, W] -> [B, 4, 128, 512] -> [B, 128, 4, 512] -> [B*128, 2048]
    xp = np.ascontiguousarray(
        x.reshape(128, 4, 128, W).transpose(0, 2, 1, 3)
    ).reshape(128, 128, 4 * W)
    ftw = _ftw_windows()
    in_maps = []
    for c in range(N_CORES):
        shard = xp[c * B_PER_CORE : (c + 1) * B_PER_CORE].reshape(
            B_PER_CORE * 128, 4 * W
        )
        in_maps.append({"image": np.ascontiguousarray(shard), "ftw": ftw})
    return in_maps


def postprocess(shards: list[np.ndarray]) -> np.ndarray:
    """Per-core uint8 0/1 outputs [B*128, 2048] -> full f32 0/255 output."""
    u8 = np.concatenate(
        [s.reshape(B_PER_CORE, 128, 4, W) for s in shards], axis=0
    )
    # [128img, 128p, 4t, 512] -> [128img, 4t, 128p, 512] -> [128, 512, 512]
    out01 = u8.transpose(0, 2, 1, 3).reshape(128, H, W, 1)
    return out01.astype(np.float32) * np.float32(255.0)


class _nullcontext:
    def __enter__(self):
        return None

    def __exit__(self, *a):
        return False


def _emit_images(nc, tc, pools, img_d, out_d, ftw):
    """One-image software pipeline: pass 2 + compare of image b-1 are emitted
    inside iteration b, so ACT's evac of image b overlaps PE's pass 2 of
    image b-1 and DVE's compares chase pass 2. PSUM is split into four
    2-bank half tiles (vtA/vtB/uA/uB x bufs=2 = 8 banks exactly) so no
    engine ever waits a full-image latency for a buffer."""
    img_pool, vt_pool, out_pool, vtps_pool, ups_pool = pools
    pending = {}

    def queue_store(b, outt):
        st = getattr(nc, STORE_ENGINE)
        item = (st, out_d[b * 128 : (b + 1) * 128, :], outt[:])
        if STORE_DELAY == 0:
            item[0].dma_start(item[1], item[2])
        else:
            pending.setdefault(b, []).append(item)

    def emit_due_store(b):
        for st, dst, src in pending.pop(b - STORE_DELAY, []):
            st.dma_start(dst, src)

    def pass1_half(img, vt_sb, half):
        """j-blocks {2*half, 2*half+1} -> one 2-bank PSUM tile -> ACT evac."""
        vt_ps = vtps_pool.tile([128, 2 * W], F32)
        for jj in range(2):
            j = 2 * half + jj
            for k in range(4):
                nc.tensor.matmul(
                    vt_ps[:, jj * 512 + WIN[k] : jj * 512 + WIN[k] + WIDTHS[k]],
                    img[:, k * 512 + j * 128 : k * 512 + j * 128 + 128],
                    ftw[:, FTW_OFF[k] : FTW_OFF[k] + WIDTHS[k]],
                    start=(k == 0),
                    stop=(k == 3),
                )
        if VARIANT in ("noact", "pe", "pe1"):
            if VARIANT != "pe1":
                # cheap substitute write so pass 2 has an allocated source
                nc.vector.memset(vt_sb[:, half * 1024 : (half + 1) * 1024], 0.0)
            return
        nc.scalar.activation(
            vt_sb[:, half * 1024 : (half + 1) * 1024], vt_ps[:],
            mybir.ActivationFunctionType.Copy,
            bias=EVAC_BIAS, scale=EVAC_SCALE,
        )

    def pass2_half(img, vt_sb, outt, half, u_full=None):
        """t-blocks {2*half, 2*half+1} -> 2-bank PSUM tile -> DVE compare."""
        if u_full is None:
            u_ps = ups_pool.tile([128, 2 * W], F32)
            base = 0
        else:
            u_ps = u_full
            base = half * 1024
        for tt in range(2):
            t = 2 * half + tt
            for k in range(4):
                nc.tensor.matmul(
                    u_ps[:, base + tt * 512 + WIN[k] : base + tt * 512 + WIN[k] + WIDTHS[k]],
                    vt_sb[:, k * 512 + t * 128 : k * 512 + t * 128 + 128],
                    ftw[:, FTW_TOTAL + FTW_OFF[k] : FTW_TOTAL + FTW_OFF[k] + WIDTHS[k]],
                    start=(k == 0),
                    stop=(k == 3),
                )
        if u_full is not None:
            return
        sl = slice(half * 1024, (half + 1) * 1024)
        if VARIANT in ("nodve", "pe"):
            nc.vector.memset(outt[:, sl], 0.0)
            return
        nc.vector.tensor_tensor(
            outt[:, sl], img[:, sl], u_ps[:], mybir.AluOpType.is_le
        )

    def tail_image(prev):
        pb, pimg, pvt = prev
        poutt = out_pool.tile([128, 4 * W], U8)
        if UPS_FULL:
            u_full = ups_pool.tile([128, 4 * W], F32)
            pass2_half(pimg, pvt, poutt, 0, u_full)
            pass2_half(pimg, pvt, poutt, 1, u_full)
            if VARIANT in ("nodve", "pe"):
                nc.vector.memset(poutt[:], 0.0)
            else:
                nc.vector.tensor_tensor(
                    poutt[:], pimg[:], u_full[:], mybir.AluOpType.is_le
                )
        else:
            pass2_half(pimg, pvt, poutt, 0)
            pass2_half(pimg, pvt, poutt, 1)
        if VARIANT != "nostore":
            queue_store(pb, poutt)

    prev = None  # (b, img, vt_sb) of the previous image
    for b in range(B_PER_CORE):
        img = img_pool.tile([128, 4 * W], F16)
        ld = getattr(nc, LOAD_ENGINE)
        ld.dma_start(img[:], img_d[b * 128 : (b + 1) * 128, :])
        emit_due_store(b)

        if VARIANT == "dma":
            outt = out_pool.tile([128, 4 * W], U8)
            nc.vector.memset(outt[:], 0.0)
            queue_store(b, outt)
            continue

        vt_sb = vt_pool.tile([128, 4 * W], F16)
        pass1_half(img, vt_sb, 0)
        pass1_half(img, vt_sb, 1)

        if VARIANT == "pe1":
            continue

        if prev is not None:
            tail_image(prev)
        prev = (b, img, vt_sb)

    if prev is not None:
        tail_image(prev)

    for b in sorted(pending):
        for st, dst, src in pending[b]:
            st.dma_start(dst, src)
    pending.clear()


def _build_nc(reps: int = 1) -> bass.Bass:
    nc = bacc.Bacc()
    img_d = nc.declare_dram_parameter(
        "image", [B_PER_CORE * 128, 4 * W], F16, isOutput=False
    )
    ftw_d = nc.declare_dram_parameter(
        "ftw", [128, 2 * FTW_TOTAL], F16, isOutput=False
    )
    out_d = nc.declare_dram_parameter(
        "out", [B_PER_CORE * 128, 4 * W], U8, isOutput=True
    )

    with tile.TileContext(nc) as tc:
        with (
            tc.tile_pool(name="const", bufs=1) as const_pool,
            tc.tile_pool(name="img", bufs=IMG_BUFS) as img_pool,
            tc.tile_pool(name="vt", bufs=VT_BUFS) as vt_pool,
            tc.tile_pool(name="outp", bufs=OUT_BUFS) as out_pool,
            tc.tile_pool(name="vtps", bufs=2, space="PSUM") as vtps_pool,
            tc.tile_pool(
                name="ups", bufs=(1 if UPS_FULL else 2), space="PSUM"
            ) as ups_pool,
        ):
            pools = (img_pool, vt_pool, out_pool, vtps_pool, ups_pool)
            ftw = const_pool.tile([128, 2 * FTW_TOTAL], F16)
            nc.sync.dma_start(ftw[:], ftw_d[:])

            if reps > 1:
                loop_ctx = tc.For_i(0, reps, 1)
            else:
                loop_ctx = _nullcontext()
            with loop_ctx:
                _emit_images(nc, tc, pools, img_d, out_d, ftw)

    nc.compile()
    return nc


_NC_CACHE = None


def _get_nc() -> bass.Bass:
    global _NC_CACHE
    if _NC_CACHE is None:
        _NC_CACHE = _build_nc()
    return _NC_CACHE


def kernel(image: np.ndarray) -> np.ndarray:
    assert image.shape == (128, H, W, 1), image.shape
    in_maps = prepare_shards(image)
    nc = _get_nc()
    res = run_bass_kernel_spmd(nc, in_maps, core_ids=list(range(N_CORES)))
    return postprocess([res.results[c]["out"] for c in range(N_CORES)])


# revision 22
# speedup vs baseline: 3.0199x; 1.1288x over previous
"""Adaptive mean thresholding (11x11 box mean, replicate border, C=2, INV)
on 8 trn2 NeuronCores. Batch data-parallel: 16 images of [512,512] per core.

HBM-traffic-minimized version (the previous f32-in/f32-out kernel was
DMA-bound at ~99us envelope for 33.6MB/core):
  - Input ships as fp16 of the *centered* image x = fp16(I - 127.5)
    (2B/px, host-side cast). Centering keeps fp16 quantization at
    ~0.01 gray-levels rms; a numpy simulation of the full quantized chain
    measures rel err 3.5e-3 vs the exact reference (gate is 2e-2).
  - Output ships as uint8 0/1 (1B/px); the host scales to 0.0/255.0 f32
    during the gather. Total traffic 12.6MB/core (~35us at 358GB/s).
  - Rows are pre-permuted on the host into the [p, (t, w)] SBUF layout
    (partition p holds rows {128t+p}), so every load descriptor is 4KB
    contiguous and every store descriptor 2KB contiguous per partition.

Algorithm per core (per image):
  Separable 11x11 box sum via two TensorE passes with the data chunk as
  lhsT (stationary) and banded FT windows as rhs; each pass transposes, so
  two passes land back in the input orientation with zero explicit
  transposes. Weights are *dyadic* scaled (F/8 and F/16, exact in fp16);
  the ACT PSUM-evacuation pass folds the exact 128/121 rescale plus the
  threshold bias (-32/11) in f32, so the final PSUM result is directly
  U = S'/121 - 2 in centered-image units and the tail is a single DVE op:
     out_u8 = (x <= U)            # 1 -> 255 case, 0 -> 0 case
  Schedule: one-image software pipeline (iteration b emits pass 1 of
  image b, then pass 2 + compare of image b-1) with PSUM split into four
  2-bank half tiles (vt x2 + u x2, bufs=2 each = 8 banks exactly), so PE
  streams matmuls back-to-back (~57 ns/MM measured, LDW fully hidden by
  FWL + the PE reorder window) while ACT evacs and DVE compares chase the
  halves. Measured ablations (shared device, medians): DMA envelope
  ~34us, loads+pass1 ~15us, all-but-compare ~29us, full ~33us quiet /
  ~40us typical. A single full-image u tile (one compare op) measures
  +20us - image b+1's pass 2 stalls on compare(b) - hence the halves.
  DMA: loads on the SP HWDGE ring; stores on the ACT HWDGE ring with
  trigger emission delayed 6 images (STORE_DELAY<=4 measures 6-12us
  slower: a store trigger whose compare hasn't finished occupies the
  8-deep strict-FIFO ACT queue and head-of-line blocks the next evac).
"""

import sys

for p in ("/opt/trn_rl_repo", "/opt/trn_rl_repo/concourse"):
    if p not in sys.path:
        sys.path.insert(0, p)

import numpy as np

import concourse.bass as bass
import concourse.bacc as bacc
import concourse.mybir as mybir
import concourse.tile as tile
from concourse.bass_utils import run_bass_kernel_spmd

F32 = mybir.dt.float32
F16 = mybir.dt.float16
U8 = mybir.dt.uint8

N_CORES = 8
B_PER_CORE = 16
H = W = 512
K = 11
PAD = K // 2

# evac: Vb = fp16((128/121) * V + BETA); U = sum F/16 * Vb = S'/121 - 2
EVAC_SCALE = 128.0 / 121.0
EVAC_BIAS = -32.0 / 11.0

IMG_BUFS = 12
VT_BUFS = 3
OUT_BUFS = 10
STORE_DELAY = 6
# "+"-separated engine cycle, indexed by image number. Splitting across
# rings measured WORSE (loads sync+scalar: +13us — load triggers on the ACT
# ring collide with evacs; gpsimd SWDGE stores: +6us — descriptor starvation).
LOAD_ENGINE = "sync"
STORE_ENGINE = "scalar"
# diagnostic ablations (timing-only; outputs are garbage for any value
# other than "full"): "dma" = loads+stores only; "pe" = loads + both PE
# passes (pass 2 reads unwritten SBUF); "pe1" = loads + pass 1 only;
# "noact" = skip evac; "nodve" = skip compares; "nostore" = skip stores.
VARIANT = "full"
# 1 = single [128,2048] u PSUM tile + one full-image DVE compare (fewer DVE
# fixed overheads, but image b+1's pass 2 must wait on compare(b));
# 0 = two [128,1024] half tiles + two compares (default).
UPS_FULL = 0

# banded windows: window k must contain the band [128k-5, 128k+133).
# k=0 banded too: start=True clears has_written for the WHOLE psum bank.
WIN = (0, 123, 251, 379)
WIDTHS = (133, 138, 138, 133)
FTW_OFF = (0, 133, 271, 409)
FTW_TOTAL = 542


def _filter_matrix() -> np.ndarray:
    """F[o, i] = number of taps of output o's clamped window hitting input i."""
    F = np.zeros((H, H), dtype=np.float64)
    for o in range(H):
        for d in range(-PAD, PAD + 1):
            F[o, min(max(o + d, 0), H - 1)] += 1.0
    return F


def _ftw_windows() -> np.ndarray:
    """[128, 2*FTW_TOTAL]: FT/8 band windows then FT/16 band windows."""
    FT = _filter_matrix().T
    out = []
    for scale in (8.0, 16.0):
        tiles = [
            FT[128 * k : 128 * (k + 1), WIN[k] : WIN[k] + WIDTHS[k]] / scale
            for k in range(4)
        ]
        out.append(np.concatenate(tiles, axis=1))
    return np.ascontiguousarray(np.concatenate(out, axis=1)).astype(np.float16)


def prepare_shards(image: np.ndarray) -> list[dict[str, np.ndarray]]:
    """Full [128, 512, 512, 1] f32 image -> per-core input maps.

    Ships x = fp16(I - 127.5) row-permuted so partition p holds image rows
    {128t + p} as 4 contiguous 512-px chunks (4KB/partition descriptors).
    """
    img = image.reshape(128, H, W).astype(np.float32)
    x = (img - np.float32(127.5)).astype(np.float16)
    # [B, H, W] -> [B, 4, 128, 512] -> [B, 128, 4, 512] -> [B*128, 2048]
    xp = np.ascontiguousarray(
        x.reshape(128, 4, 128, W).transpose(0, 2, 1, 3)
    ).reshape(128, 128, 4 * W)
    ftw = _ftw_windows()
    in_maps = []
    for c in range(N_CORES):
        shard = xp[c * B_PER_CORE : (c + 1) * B_PER_CORE].reshape(
            B_PER_CORE * 128, 4 * W
        )
        in_maps.append({"image": np.ascontiguousarray(shard), "ftw": ftw})
    return in_maps


def postprocess(shards: list[np.ndarray]) -> np.ndarray:
    """Per-core uint8 0/1 outputs [B*128, 2048] -> full f32 0/255 output."""
    u8 = np.concatenate(
        [s.reshape(B_PER_CORE, 128, 4, W) for s in shards], axis=0
    )
    # [128img, 128p, 4t, 512] -> [128img, 4t, 128p, 512] -> [128, 512, 512]
    out01 = u8.transpose(0, 2, 1, 3).reshape(128, H, W, 1)
    return out01.astype(np.float32) * np.float32(255.0)


class _nullcontext:
    def __enter__(self):
        return None

    def __exit__(self, *a):
        return False


def _emit_images(nc, tc, pools, img_d, out_d, ftw):
    """One-image software pipeline: pass 2 + compare of image b-1 are emitted
    inside iteration b, so ACT's evac of image b overlaps PE's pass 2 of
    image b-1 and DVE's compares chase pass 2. PSUM is split into four
    2-bank half tiles (vtA/vtB/uA/uB x bufs=2 = 8 banks exactly) so no
    engine ever waits a full-image latency for a buffer."""
    img_pool, vt_pool, out_pool, vtps_pool, ups_pool = pools
    pending = {}

    store_cycle = STORE_ENGINE.split("+")
    load_cycle = LOAD_ENGINE.split("+")

    def queue_store(b, outt):
        st = getattr(nc, store_cycle[b % len(store_cycle)])
        item = (st, out_d[b * 128 : (b + 1) * 128, :], outt[:])
        if STORE_DELAY == 0:
            item[0].dma_start(item[1], item[2])
        else:
            pending.setdefault(b, []).append(item)

    def emit_due_store(b):
        for st, dst, src in pending.pop(b - STORE_DELAY, []):
            st.dma_start(dst, src)

    def pass1_half(img, vt_sb, half):
        """j-blocks {2*half, 2*half+1} -> one 2-bank PSUM tile -> ACT evac."""
        vt_ps = vtps_pool.tile([128, 2 * W], F32)
        for jj in range(2):
            j = 2 * half + jj
            for k in range(4):
                nc.tensor.matmul(
                    vt_ps[:, jj * 512 + WIN[k] : jj * 512 + WIN[k] + WIDTHS[k]],
                    img[:, k * 512 + j * 128 : k * 512 + j * 128 + 128],
                    ftw[:, FTW_OFF[k] : FTW_OFF[k] + WIDTHS[k]],
                    start=(k == 0),
                    stop=(k == 3),
                )
        if VARIANT in ("noact", "pe", "pe1"):
            if VARIANT != "pe1":
                # cheap substitute write so pass 2 has an allocated source
                nc.vector.memset(vt_sb[:, half * 1024 : (half + 1) * 1024], 0.0)
            return
        nc.scalar.activation(
            vt_sb[:, half * 1024 : (half + 1) * 1024], vt_ps[:],
            mybir.ActivationFunctionType.Copy,
            bias=EVAC_BIAS, scale=EVAC_SCALE,
        )

    def pass2_half(img, vt_sb, outt, half, u_full=None):
        """t-blocks {2*half, 2*half+1} -> 2-bank PSUM tile -> DVE compare."""
        if u_full is None:
            u_ps = ups_pool.tile([128, 2 * W], F32)
            base = 0
        else:
            u_ps = u_full
            base = half * 1024
        for tt in range(2):
            t = 2 * half + tt
            for k in range(4):
                nc.tensor.matmul(
                    u_ps[:, base + tt * 512 + WIN[k] : base + tt * 512 + WIN[k] + WIDTHS[k]],
                    vt_sb[:, k * 512 + t * 128 : k * 512 + t * 128 + 128],
                    ftw[:, FTW_TOTAL + FTW_OFF[k] : FTW_TOTAL + FTW_OFF[k] + WIDTHS[k]],
                    start=(k == 0),
                    stop=(k == 3),
                )
        if u_full is not None:
            return
        sl = slice(half * 1024, (half + 1) * 1024)
        if VARIANT in ("nodve", "pe"):
            nc.vector.memset(outt[:, sl], 0.0)
            return
        nc.vector.tensor_tensor(
            outt[:, sl], img[:, sl], u_ps[:], mybir.AluOpType.is_le
        )

    def tail_image(prev):
        pb, pimg, pvt = prev
        poutt = out_pool.tile([128, 4 * W], U8)
        if UPS_FULL:
            u_full = ups_pool.tile([128, 4 * W], F32)
            pass2_half(pimg, pvt, poutt, 0, u_full)
            pass2_half(pimg, pvt, poutt, 1, u_full)
            if VARIANT in ("nodve", "pe"):
                nc.vector.memset(poutt[:], 0.0)
            else:
                nc.vector.tensor_tensor(
                    poutt[:], pimg[:], u_full[:], mybir.AluOpType.is_le
                )
        else:
            pass2_half(pimg, pvt, poutt, 0)
            pass2_half(pimg, pvt, poutt, 1)
        if VARIANT != "nostore":
            queue_store(pb, poutt)

    prev = None  # (b, img, vt_sb) of the previous image
    for b in range(B_PER_CORE):
        img = img_pool.tile([128, 4 * W], F16)
        ld = getattr(nc, load_cycle[b % len(load_cycle)])
        ld.dma_start(img[:], img_d[b * 128 : (b + 1) * 128, :])
        emit_due_store(b)

        if VARIANT == "dma":
            outt = out_pool.tile([128, 4 * W], U8)
            nc.vector.memset(outt[:], 0.0)
            queue_store(b, outt)
            continue

        vt_sb = vt_pool.tile([128, 4 * W], F16)
        pass1_half(img, vt_sb, 0)
        pass1_half(img, vt_sb, 1)

        if VARIANT == "pe1":
            continue

        if prev is not None:
            tail_image(prev)
        prev = (b, img, vt_sb)

    if prev is not None:
        tail_image(prev)

    for b in sorted(pending):
        for st, dst, src in pending[b]:
            st.dma_start(dst, src)
    pending.clear()


def _build_nc(reps: int = 1) -> bass.Bass:
    nc = bacc.Bacc()
    img_d = nc.declare_dram_parameter(
        "image", [B_PER_CORE * 128, 4 * W], F16, isOutput=False
    )
    ftw_d = nc.declare_dram_parameter(
        "ftw", [128, 2 * FTW_TOTAL], F16, isOutput=False
    )
    out_d = nc.declare_dram_parameter(
        "out", [B_PER_CORE * 128, 4 * W], U8, isOutput=True
    )

    with tile.TileContext(nc) as tc:
        with (
            tc.tile_pool(name="const", bufs=1) as const_pool,
            tc.tile_pool(name="img", bufs=IMG_BUFS) as img_pool,
            tc.tile_pool(name="vt", bufs=VT_BUFS) as vt_pool,
            tc.tile_pool(name="outp", bufs=OUT_BUFS) as out_pool,
            tc.tile_pool(name="vtps", bufs=2, space="PSUM") as vtps_pool,
            tc.tile_pool(
                name="ups", bufs=(1 if UPS_FULL else 2), space="PSUM"
            ) as ups_pool,
        ):
            pools = (img_pool, vt_pool, out_pool, vtps_pool, ups_pool)
            ftw = const_pool.tile([128, 2 * FTW_TOTAL], F16)
            nc.sync.dma_start(ftw[:], ftw_d[:])

            if reps > 1:
                loop_ctx = tc.For_i(0, reps, 1)
            else:
                loop_ctx = _nullcontext()
            with loop_ctx:
                _emit_images(nc, tc, pools, img_d, out_d, ftw)

    nc.compile()
    return nc


_NC_CACHE = None


def _get_nc() -> bass.Bass:
    global _NC_CACHE
    if _NC_CACHE is None:
        _NC_CACHE = _build_nc()
    return _NC_CACHE


def kernel(image: np.ndarray) -> np.ndarray:
    assert image.shape == (128, H, W, 1), image.shape
    in_maps = prepare_shards(image)
    nc = _get_nc()
    res = run_bass_kernel_spmd(nc, in_maps, core_ids=list(range(N_CORES)))
    return postprocess([res.results[c]["out"] for c in range(N_CORES)])
